# revision 33
# baseline (speedup 1.0000x reference)
"""Bidirectional margin-ranking loss on 8 Trainium2 NeuronCores.

Math per row-unit n of all_rows = [S; S.T] ([1024, 512] with 0/1 labels):
  tot_n = sum_{i in pos, j in neg} relu(margin + S[n,j] - S[n,i])
  mean_n = tot_n / (npos_n*nneg_n); result = sum(mean) / sum(valid).

Host layout prep (pure sorting/permutation): per row, choose the pivot role
(positives vs negated negatives - relu(a-b) == relu((-b)-(-a))) minimizing
total need, then sort pivots ascending and the stream descending (fp16).
need(pivot) = #{stream > pivot}.  The deepest K_SC pivots per row go to the
Scalar engine (relu activation, bias=-pivot, accum per column).  The rest go
to the Vector engine in groups of 4 via a custom paged DVE op (RANK_PGMAX4,
uop FSM built below): one instruction streams S groups x N-prefix of the
stream ([P,S,N] stride-0 broadcast AP), re-latching 4 pivots per page from
the in1 stream into swap flops and accumulating
  sum max(a_j, b_p) + imm2 * sum b_p   (imm2 = -N)
== sum relu(a_j - b_p) exactly, because beyond each group's prefix all
a <= pivot (guaranteed by the strata DP over the need envelope
m(t) = max_rows #{needs > t}).  Strata write their RUNNING accumulator to a
reversed out AP so each stratum's final element lands at a fixed column of
one buffer - no accumulator reads, no inter-instruction fencing; strata
issue back-to-back.  A single reduce_sum produces per-row totals, DMA'd out;
the host applies the 1/(npos*nneg) weights and the final division.
"""

import numpy as np
from operator import add as _operator_add

import concourse.bacc as bacc
import concourse.dve_ops as dve_ops
import concourse.mybir as mybir
from concourse.bass_utils import run_bass_kernel_spmd
from concourse.dve_spec import C0, C1, C3, Spec, Src0, Zero, _spill_c3_to_src1, maxx
from concourse.dve_uop import (
    DISABLE,
    ENABLE,
    AluInp,
    AluOp,
    DelayInp,
    DveOpSpec,
    InpSel,
    OutPath,
    OutSel,
    Trigger,
    UopConfig,
)

F32 = mybir.dt.float32
F16 = mybir.dt.float16
ALU = mybir.AluOpType
AF = mybir.ActivationFunctionType

MARGIN = 0.2
LBIG = 12.0
B = 512
R = 512
P = 128
N_CORES = 8
K_SC = 15          # pivots per row handled by the Scalar engine
C_INSTR = 100.0    # cycles of fixed overhead per paged-DVE instruction (DP)

_CACHE = {}

# --------------------------------------------------------------------------
# custom paged DVE op


def _seed_uop(carry):
    u = UopConfig()
    u.enable_input(InpSel.ZERO, 1)
    for j in range(8):
        u.datapath_config[j].pass_through_alu()
        u.datapath_config[j].pass_through_delay(0)
    if not carry:
        d7 = u.datapath_config[7]
        d7.enable_alu(AluOp.BYPASS, AluInp.PREV_DELAY_0, AluInp.PREV_DELAY_0)
        d7.alu_out_a_enable = ENABLE
    else:
        # bubble must not touch the stage-7 accumulator flop
        u.datapath_config[7].alu_out_enable = DISABLE
    u.require_inp0 = DISABLE
    u.require_inp1 = DISABLE
    u.trigger = (Trigger.COUNT, Trigger.NONE, Trigger.NONE)
    u.repeat_count = 1
    u.next_uop = (1, 0, 0)
    u.accum_enabled = ENABLE
    return u


def _latch_uop(target, nxt):
    u = UopConfig()
    u.enable_input(InpSel.SRC_1, 1)    # lane0 = pivot
    u.enable_input(InpSel.CONST_2, 2)  # lane1 = imm2 = -N
    for j in range(8):
        u.datapath_config[j].pass_through_alu()
        if j < 6:
            u.datapath_config[j].pass_through_delay(0, 1)
    t = u.datapath_config[target]
    t.op = AluOp.BYPASS
    t.alu_src0 = AluInp.PREV_DELAY_0
    t.alu_src1 = AluInp.PREV_DELAY_0
    t.swap_enable = ENABLE
    d6 = u.datapath_config[6]
    d6.enable_alu(AluOp.MULTIPLY, AluInp.PREV_DELAY_0, AluInp.PREV_DELAY_1)
    d7 = u.datapath_config[7]
    d7.enable_alu(AluOp.ADD, AluInp.CURR_ALU_OUT, AluInp.PREV_ALU_OUT)
    d7.alu_out_a_enable = ENABLE
    u.require_inp0 = DISABLE
    u.require_inp1 = ENABLE
    u.trigger = (Trigger.COUNT, Trigger.NONE, Trigger.NONE)
    u.repeat_count = 1
    u.next_uop = (nxt, 0, 0)
    u.accum_enabled = ENABLE
    return u


def _steady_uop(first_latch):
    u = UopConfig()
    u.enable_input(InpSel.SRC_0, 1)  # lane0 = x
    dp = u.datapath_config
    dp[0].enable_alu(AluOp.MAX, AluInp.PREV_DELAY_0, AluInp.CURR_SWAP_OUT)
    dp[0].pass_through_delay(0)
    dp[1].enable_alu(AluOp.MAX, AluInp.PREV_DELAY_0, AluInp.CURR_SWAP_OUT)
    dp[1].pass_through_delay(0)
    dp[1].enable_delay_from_src(DelayInp.PREV_ALU_OUT, 1)
    dp[2].enable_alu(AluOp.ADD, AluInp.PREV_ALU_OUT, AluInp.PREV_DELAY_1)
    dp[2].pass_through_delay(0)
    dp[3].enable_alu(AluOp.MAX, AluInp.PREV_DELAY_0, AluInp.CURR_SWAP_OUT)
    dp[3].pass_through_delay(0)
    dp[3].enable_delay_from_src(DelayInp.PREV_ALU_OUT, 1)
    dp[4].enable_alu(AluOp.MAX, AluInp.PREV_DELAY_0, AluInp.CURR_SWAP_OUT)
    dp[4].pass_through_delay(1)
    dp[4].enable_delay_from_src(DelayInp.PREV_ALU_OUT, 2)
    dp[5].enable_alu(AluOp.ADD, AluInp.PREV_ALU_OUT, AluInp.PREV_DELAY_2)
    dp[5].pass_through_delay(1)
    dp[6].enable_alu(AluOp.ADD, AluInp.PREV_ALU_OUT, AluInp.PREV_DELAY_1)
    dp[7].enable_alu(AluOp.ADD, AluInp.CURR_ALU_OUT, AluInp.PREV_ALU_OUT)
    dp[7].alu_out_a_enable = ENABLE
    u.enable_output(OutSel.ALU_OUT, OutPath.WR0_LO)
    u.require_inp0 = ENABLE
    u.require_inp1 = DISABLE
    u.trigger = (Trigger.SRC_TENSOR_DONE, Trigger.SUB_DIM_DONE, Trigger.NONE)
    u.next_uop = (0, first_latch, 0)
    u.accum_enabled = ENABLE
    return u


def _build_pg_uops(name, ver, carry=False):
    assert ver == "v3"
    uops = [_seed_uop(carry)]
    for k, t in enumerate((0, 1, 3, 4)):
        uops.append(_latch_uop(t, nxt=2 + k if k < 3 else 5))
    uops.append(_steady_uop(first_latch=1))
    for u in uops:
        u.validate(ver)
    return DveOpSpec(name=name, opcode=dve_ops.get_dve_sub_opcode(name),
                     uops=uops, rd1_en=True)


class _HandOp:
    def __init__(self, name, spec, build, subdim):
        self.name = name
        self.spec = spec
        self.subdim = subdim
        self._build = build
        self._compiled = {}

    def compile(self, ver):
        if ver not in self._compiled:
            self._compiled[ver] = self._build(self.name, ver)
        return self._compiled[ver]


def _pg_reference(in0, in1, c0, c1, c2):
    Pp = in0.shape[0]
    S = in1.shape[-1] // 4
    x = in0.reshape(Pp, S, -1).astype(np.float32)
    bb = in1.reshape(Pp, S, 4).astype(np.float32)
    m = np.maximum(x[:, :, None, :], bb[:, :, :, None])
    acc = m.sum(axis=(1, 2, 3)) + c2 * bb.reshape(Pp, -1).sum(axis=1)
    return np.zeros((Pp, 1), np.float32), acc.reshape(Pp, 1)


def _register_pg_ops():
    names = ("RANK_PGMAX4", "RANK_PGMAX4C")
    if names[0] in _CACHE:
        return tuple(_CACHE[n] for n in names)
    if names[0] in dve_ops._SUB_OPCODE_FOR_NAME:
        for n in names:
            _CACHE[n] = next(o for o in dve_ops.OPS if o.name == n)
        return tuple(_CACHE[n] for n in names)
    meta = Spec(
        body=_spill_c3_to_src1(maxx(Src0, C0) + maxx(Src0, C1) + maxx(Src0, C3)),
        accum=_operator_add, accum_init=Zero, reference=_pg_reference)
    ops = []
    for name, carry in ((names[0], False), (names[1], True)):
        op = _HandOp(name, meta,
                     (lambda n, v, c=carry: _build_pg_uops(n, v, carry=c)),
                     subdim=True)
        row = 1 + len(dve_ops.OPS)
        assert row < 0x20
        dve_ops.OPS.append(op)
        dve_ops.CUSTOM_DVE_SPECS[op.name] = op.spec
        dve_ops._SUB_OPCODE_FOR_NAME[op.name] = row
        _CACHE[name] = op
        ops.append(op)
    return tuple(ops)


# --------------------------------------------------------------------------
# host-side layout prep


def _prepare(scores, labels):
    """Sort/compact all 1024 row-units. Returns dict of per-row arrays and
    the shared strata plan."""
    scores = np.ascontiguousarray(np.asarray(scores), dtype=np.float32)
    lab = np.ascontiguousarray(np.asarray(labels)).astype(np.float32)
    all_s = np.concatenate([scores, scores.T], axis=0)
    all_l = np.concatenate([lab, lab.T], axis=0)
    pos = all_l > 0.5
    rows = all_s.shape[0]

    npos = pos.sum(axis=1)
    nneg = all_s.shape[1] - npos
    wn = int(max(nneg.max(), npos.max()))

    a_desc = np.full((rows, wn), -LBIG, dtype=np.float16)
    b_list = []      # per row: fp16 pivots ascending (scalar K first removed)
    needs_list = []  # per row: needs of the DVE pivots (non-increasing)
    sc_needs = np.zeros((rows, K_SC), dtype=np.int64)
    bsc = np.full((rows, K_SC), LBIG, dtype=np.float32)

    for r in range(rows):
        # role A: stream = negatives+margin desc, pivots = positives asc
        avA = np.sort((all_s[r][~pos[r]] + MARGIN).astype(np.float16))
        bvA = np.sort(all_s[r][pos[r]].astype(np.float16))
        ndA = len(avA) - np.searchsorted(avA, bvA, side="right")
        # role B: stream = -positives desc, pivots = -(neg+margin) asc
        # (relu(a_j - b_i) == relu((-b_i) - (-a_j)))
        avB = np.sort((-all_s[r][pos[r]]).astype(np.float16))
        bvB = np.sort((-(all_s[r][~pos[r]] + MARGIN)).astype(np.float16))
        ndB = len(avB) - np.searchsorted(avB, bvB, side="right")
        sA = np.sort(ndA[ndA > 0])[::-1][K_SC:].sum()
        sB = np.sort(ndB[ndB > 0])[::-1][K_SC:].sum()
        av, bv, need = (avA, bvA, ndA) if sA <= sB else (avB, bvB, ndB)
        a_desc[r, :len(av)] = av[::-1]
        order = np.argsort(need, kind="stable")[::-1]  # deepest first
        bv, need = bv[order], need[order]
        k = min(K_SC, len(bv))
        bsc[r, :k] = -bv[:k].astype(np.float32)
        sc_needs[r, :k] = need[:k]
        bd, nd = bv[k:], need[k:]
        nz = nd > 0
        b_list.append(bd[nz])
        needs_list.append(nd[nz])

    # envelope m(t) = max over rows of #{DVE needs > t}
    t_arr = np.arange(wn + 1)
    m = np.zeros(wn + 1, dtype=np.int64)
    for nd in needs_list:
        if len(nd):
            cnt = (nd[:, None] > t_arr[None, :]).sum(0)
            np.maximum(m, cnt, out=m)

    # threshold DP -> strata [(S_groups, N_len)] in descending-N order
    g = np.ceil(m / 4.0).astype(np.int64)
    INF = float("inf")
    dp = np.full(wn + 1, INF)
    dp[wn] = 0.0
    parent = np.full(wn + 1, -1, dtype=np.int64)
    for t in range(wn - 1, -1, -1):
        best, bu = INF, -1
        for u in range(t + 1, wn + 1):
            if dp[u] == INF:
                continue
            ag = g[t] - g[u]
            c = dp[u] + ag * (u + 4) + (C_INSTR if ag > 0 else 0.0)
            if c < best:
                best, bu = c, u
        dp[t] = best
        parent[t] = bu
    strata = []  # descending N
    t = 0
    chain = []
    while t < wn and parent[t] != -1:
        u = parent[t]
        ag = int(g[t] - g[u])
        if ag:
            chain.append((ag, int(u)))
        t = u
    strata = chain[::-1]  # largest N first (covers deepest ranks)
    G = sum(s for s, _ in strata)

    # pack the DVE pivot stream rank-major (deepest first), pad with +LBIG
    b_rank = np.full((rows, 4 * G), LBIG, dtype=np.float16)
    for r in range(rows):
        bd = b_list[r]
        b_rank[r, :len(bd)] = bd

    # permute pivot columns into execution order (ascending N strata)
    offs_desc = np.cumsum([0] + [si for si, _ in strata])[:-1]
    exec_order = sorted(zip(strata, offs_desc), key=lambda z: z[0][1])
    perm = []
    exec_strata = []  # (S, N) ascending-N with contiguous exec layout
    for (si, ni), od in exec_order:
        perm.extend(range(4 * od, 4 * (od + si)))
        exec_strata.append((si, ni))
    b_dve = np.ascontiguousarray(b_rank[:, perm])

    # scalar column stream lengths (envelope over rows)
    sc_len = sc_needs.max(axis=0)  # [K_SC]

    cnt = (npos * nneg).astype(np.float64)
    valid = cnt > 0
    w = np.where(valid, 1.0 / np.maximum(cnt, 1.0), 0.0)
    aux = np.stack([w, valid.astype(np.float64)], axis=1).astype(np.float32)

    return dict(a=a_desc, b=b_dve, bsc=bsc, aux=aux, wn=wn, G=G,
                strata=tuple(exec_strata),
                sc_len=tuple(int(x) for x in sc_len))


# --------------------------------------------------------------------------
# device program


def _build_program(wn, G, strata, sc_len, carry=False, debug=False):
    """strata: exec-ordered (ascending N), contiguous column layout.

    Strata write their running accumulator to a reversed out AP so the final
    element of stratum idx lands at acct[:, K+idx]; no accum_out / reads /
    staircase needed.  Scalar accums land at acct[:, 0:K].  One reduce."""
    key = ("pg", wn, G, tuple(strata), tuple(sc_len), carry, debug)
    if key in _CACHE:
        return _CACHE[key]
    pg, pgc = _register_pg_ops()

    offs = []
    o = 0
    for si, ni in strata:
        offs.append(o)
        o += si
    exec_order = list(zip(strata, offs))
    n_str = len(strata)
    n_chunk = min(3, n_str)                     # strata covered by chunk 1
    n_first = exec_order[n_chunk - 1][0][1]     # a-prefix needed by them
    n_zero = exec_order[0][0][1]                # stratum-1 a-prefix
    first_S = sum(si for (si, _), _ in exec_order[:n_chunk])
    max_sn = max(si * ni for si, ni in strata)
    K = len(sc_len)
    AW = K + n_str + max_sn                     # acct width

    nc = bacc.Bacc("TRN2", target_bir_lowering=False, debug=False,
                   num_devices=N_CORES)
    a_in = nc.dram_tensor("a_blk", [P, wn], F16, kind="ExternalInput").ap()
    b_in = nc.dram_tensor("b_blk", [P, 4 * G], F16, kind="ExternalInput").ap()
    c_in = nc.dram_tensor("bsc_blk", [P, K], F32, kind="ExternalInput").ap()
    out = nc.dram_tensor("out", [P, 1], F32, kind="ExternalOutput").ap()

    a = nc.alloc_sbuf_tensor("a", [P, wn], F16).ap()
    b = nc.alloc_sbuf_tensor("b", [P, 4 * G], F16).ap()
    bsc = nc.alloc_sbuf_tensor("bsc", [P, K], F32).ap()
    acct = nc.alloc_sbuf_tensor("acct", [P, AW], F32).ap()
    trash_s = nc.alloc_sbuf_tensor("trash_s", [P, 1], F32).ap()
    tot = nc.alloc_sbuf_tensor("tot", [P, 1], F32).ap()

    with (
        nc.psum_tensor("trash_ps", [P, wn], F32) as trash_ps_h,
        nc.semaphore("s_a0") as s_a0,
        nc.semaphore("s_a1") as s_a1,
        nc.semaphore("s_a2") as s_a2,
        nc.semaphore("s_b1") as s_b1,
        nc.semaphore("s_b2") as s_b2,
        nc.semaphore("s_bsc") as s_bsc,
        nc.semaphore("s_act") as s_act,
        nc.semaphore("s_vv") as s_vv,
        nc.semaphore("s_v") as s_v,
        nc.Block() as block,
    ):
        @block.sync
        def _(sync):
            sync.dma_start(a[:, :n_zero], a_in[:, :n_zero]).then_inc(s_a0, 16)
            if n_first > n_zero:
                sync.dma_start(a[:, n_zero:n_first],
                               a_in[:, n_zero:n_first]).then_inc(s_a1, 16)
            else:
                sync.sem_inc(s_a1, 16)
            if wn > n_first:
                sync.dma_start(a[:, n_first:],
                               a_in[:, n_first:]).then_inc(s_a2, 16)
            else:
                sync.sem_inc(s_a2, 16)
            sync.wait_ge(s_v, 1)
            sync.dma_start(out[:], tot[:]).then_inc(s_v, 16)

        @block.gpsimd
        def _(gpsimd):
            if G > first_S:
                gpsimd.dma_start(b[:, 4 * first_S:],
                                 b_in[:, 4 * first_S:]).then_inc(s_b2, 16)
            else:
                gpsimd.sem_inc(s_b2, 16)

        @block.scalar
        def _(scalar):
            scalar.dma_start(b[:, :4 * first_S],
                             b_in[:, :4 * first_S]).then_inc(s_b1, 16)
            scalar.dma_start(bsc[:], c_in[:]).then_inc(s_bsc, 16)
            trash_ps = trash_ps_h.ap()
            # warm the Relu table while DMAs fly
            zc = nc.const_aps.scalar_like(0.0, trash_s[:, 0:1])
            scalar.activation(trash_s[:, 0:1], zc, AF.Relu, bias=0.0, scale=1.0)
            scalar.wait_ge(s_bsc, 16)
            scalar.wait_ge(s_a2, 16)
            for j in range(K):
                if j:
                    scalar.wait_ge(s_act, j)
                L = max(sc_len[j], 1)
                scalar.activation(trash_ps[:, :L], a[:, :L], AF.Relu,
                                  bias=bsc[:, j:j + 1], scale=1.0,
                                  accum_out=acct[:, j:j + 1]).then_inc(s_act, 1)

        @block.vector
        def _(vector):
            vector.wait_ge(s_a0, 16)
            vector.wait_ge(s_b1, 16)
            done_a1 = False
            done_a2 = False
            last = None
            for idx, ((S, N), off) in enumerate(exec_order):
                if idx == n_chunk:
                    vector.wait_ge(s_b2, 16)
                if not done_a1 and N > n_zero:
                    vector.wait_ge(s_a1, 16)
                    done_a1 = True
                if not done_a2 and N > n_first:
                    vector.wait_ge(s_a2, 16)
                    done_a2 = True
                in0 = a[:, :N].unsqueeze(1).broadcast_to((P, S, N))
                in1 = b[:, 4 * off:4 * (off + S)]
                hi = K + idx + S * N - 1
                last = vector._custom_dve(
                    pg, out=acct[:, hi:K + idx - 1:-1], in0=in0, in1=in1,
                    s0=0.0, s1=0.0, imm2=-float(N))
            last.then_inc(s_vv, 1)
            vector.wait_ge(s_vv, 1)
            vector.wait_ge(s_act, K)
            vector.reduce_sum(tot[:], acct[:, :K + n_str],
                              axis=mybir.AxisListType.X).then_inc(s_v, 1)

    nc.compile()
    _CACHE[key] = nc
    return nc


# --------------------------------------------------------------------------


def kernel(scores, labels):
    prep = _prepare(scores, labels)
    nc = _build_program(prep["wn"], prep["G"], tuple(prep["strata"]),
                        prep["sc_len"], carry=False)
    in_maps = [{"a_blk": prep["a"][P * k:P * (k + 1)],
                "b_blk": prep["b"][P * k:P * (k + 1)],
                "bsc_blk": prep["bsc"][P * k:P * (k + 1)]}
               for k in range(N_CORES)]
    res = run_bass_kernel_spmd(nc, in_maps, list(range(N_CORES)))
    tots = np.concatenate([res.results[k]["out"][:, 0]
                           for k in range(N_CORES)])
    w = prep["aux"][:, 0].astype(np.float64)
    valid = prep["aux"][:, 1].astype(np.float64)
    return np.float32((tots * w).sum() / valid.sum())


# revision 34
# speedup vs baseline: 1.0755x; 1.0755x over previous
"""Bidirectional margin-ranking loss on 8 Trainium2 NeuronCores.

Math per row-unit n of all_rows = [S; S.T] ([1024, 512] with 0/1 labels):
  tot_n = sum_{i in pos, j in neg} relu(margin + S[n,j] - S[n,i])
  mean_n = tot_n / (npos_n*nneg_n); result = sum(mean) / sum(valid).

Host layout prep (pure sorting/permutation): per row, choose the pivot role
(positives vs negated negatives - relu(a-b) == relu((-b)-(-a))) minimizing
total need, then sort pivots ascending and the stream descending (fp16).
need(pivot) = #{stream > pivot}.  The deepest K_SC pivots per row go to the
Scalar engine (relu activation, bias=-pivot, accum per column).  The rest go
to the Vector engine in groups of 4 via a custom paged DVE op (RANK_PGMAX4,
uop FSM built below): one instruction streams S groups x N-prefix of the
stream ([P,S,N] stride-0 broadcast AP), re-latching 4 pivots per page from
the in1 stream into swap flops and accumulating
  sum max(a_j, b_p) + imm2 * sum b_p   (imm2 = -N)
== sum relu(a_j - b_p) exactly, because beyond each group's prefix all
a <= pivot (guaranteed by the strata DP over the need envelope
m(t) = max_rows #{needs > t}).  Strata write their RUNNING accumulator to a
reversed out AP so each stratum's final element lands at a fixed column of
one buffer - no accumulator reads, no inter-instruction fencing; strata
issue back-to-back.  A single reduce_sum produces per-row totals, DMA'd out;
the host applies the 1/(npos*nneg) weights and the final division.
"""

import numpy as np
from operator import add as _operator_add

import concourse.bacc as bacc
import concourse.dve_ops as dve_ops
import concourse.mybir as mybir
from concourse.bass_utils import run_bass_kernel_spmd
from concourse.dve_spec import C0, C1, C3, Spec, Src0, Zero, _spill_c3_to_src1, maxx
from concourse.dve_uop import (
    DISABLE,
    ENABLE,
    AluInp,
    AluOp,
    DelayInp,
    DveOpSpec,
    InpSel,
    OutPath,
    OutSel,
    Trigger,
    UopConfig,
)

F32 = mybir.dt.float32
F16 = mybir.dt.float16
ALU = mybir.AluOpType
AF = mybir.ActivationFunctionType

MARGIN = 0.2
LBIG = 12.0
B = 512
R = 512
P = 128
N_CORES = 8
K_SC = 14          # pivots per row handled by the Scalar engine
C_INSTR = 100.0    # cycles of fixed overhead per paged-DVE instruction (DP)

_CACHE = {}

# --------------------------------------------------------------------------
# custom paged DVE op


def _seed_uop(carry):
    u = UopConfig()
    u.enable_input(InpSel.ZERO, 1)
    for j in range(8):
        u.datapath_config[j].pass_through_alu()
        u.datapath_config[j].pass_through_delay(0)
    if not carry:
        d7 = u.datapath_config[7]
        d7.enable_alu(AluOp.BYPASS, AluInp.PREV_DELAY_0, AluInp.PREV_DELAY_0)
        d7.alu_out_a_enable = ENABLE
    else:
        # bubble must not touch the stage-7 accumulator flop
        u.datapath_config[7].alu_out_enable = DISABLE
    u.require_inp0 = DISABLE
    u.require_inp1 = DISABLE
    u.trigger = (Trigger.COUNT, Trigger.NONE, Trigger.NONE)
    u.repeat_count = 1
    u.next_uop = (1, 0, 0)
    u.accum_enabled = ENABLE
    return u


def _latch_uop(target, nxt):
    u = UopConfig()
    u.enable_input(InpSel.SRC_1, 1)    # lane0 = pivot
    u.enable_input(InpSel.CONST_2, 2)  # lane1 = imm2 = -N
    for j in range(8):
        u.datapath_config[j].pass_through_alu()
        if j < 6:
            u.datapath_config[j].pass_through_delay(0, 1)
    t = u.datapath_config[target]
    t.op = AluOp.BYPASS
    t.alu_src0 = AluInp.PREV_DELAY_0
    t.alu_src1 = AluInp.PREV_DELAY_0
    t.swap_enable = ENABLE
    d6 = u.datapath_config[6]
    d6.enable_alu(AluOp.MULTIPLY, AluInp.PREV_DELAY_0, AluInp.PREV_DELAY_1)
    d7 = u.datapath_config[7]
    d7.enable_alu(AluOp.ADD, AluInp.CURR_ALU_OUT, AluInp.PREV_ALU_OUT)
    d7.alu_out_a_enable = ENABLE
    u.require_inp0 = DISABLE
    u.require_inp1 = ENABLE
    u.trigger = (Trigger.COUNT, Trigger.NONE, Trigger.NONE)
    u.repeat_count = 1
    u.next_uop = (nxt, 0, 0)
    u.accum_enabled = ENABLE
    return u


def _steady_uop(first_latch):
    u = UopConfig()
    u.enable_input(InpSel.SRC_0, 1)  # lane0 = x
    dp = u.datapath_config
    dp[0].enable_alu(AluOp.MAX, AluInp.PREV_DELAY_0, AluInp.CURR_SWAP_OUT)
    dp[0].pass_through_delay(0)
    dp[1].enable_alu(AluOp.MAX, AluInp.PREV_DELAY_0, AluInp.CURR_SWAP_OUT)
    dp[1].pass_through_delay(0)
    dp[1].enable_delay_from_src(DelayInp.PREV_ALU_OUT, 1)
    dp[2].enable_alu(AluOp.ADD, AluInp.PREV_ALU_OUT, AluInp.PREV_DELAY_1)
    dp[2].pass_through_delay(0)
    dp[3].enable_alu(AluOp.MAX, AluInp.PREV_DELAY_0, AluInp.CURR_SWAP_OUT)
    dp[3].pass_through_delay(0)
    dp[3].enable_delay_from_src(DelayInp.PREV_ALU_OUT, 1)
    dp[4].enable_alu(AluOp.MAX, AluInp.PREV_DELAY_0, AluInp.CURR_SWAP_OUT)
    dp[4].pass_through_delay(1)
    dp[4].enable_delay_from_src(DelayInp.PREV_ALU_OUT, 2)
    dp[5].enable_alu(AluOp.ADD, AluInp.PREV_ALU_OUT, AluInp.PREV_DELAY_2)
    dp[5].pass_through_delay(1)
    dp[6].enable_alu(AluOp.ADD, AluInp.PREV_ALU_OUT, AluInp.PREV_DELAY_1)
    dp[7].enable_alu(AluOp.ADD, AluInp.CURR_ALU_OUT, AluInp.PREV_ALU_OUT)
    dp[7].alu_out_a_enable = ENABLE
    u.enable_output(OutSel.ALU_OUT, OutPath.WR0_LO)
    u.require_inp0 = ENABLE
    u.require_inp1 = DISABLE
    u.trigger = (Trigger.SRC_TENSOR_DONE, Trigger.SUB_DIM_DONE, Trigger.NONE)
    u.next_uop = (0, first_latch, 0)
    u.accum_enabled = ENABLE
    return u


def _build_pg_uops(name, ver, carry=False):
    assert ver == "v3"
    uops = [_seed_uop(carry)]
    for k, t in enumerate((0, 1, 3, 4)):
        uops.append(_latch_uop(t, nxt=2 + k if k < 3 else 5))
    uops.append(_steady_uop(first_latch=1))
    for u in uops:
        u.validate(ver)
    return DveOpSpec(name=name, opcode=dve_ops.get_dve_sub_opcode(name),
                     uops=uops, rd1_en=True)


class _HandOp:
    def __init__(self, name, spec, build, subdim):
        self.name = name
        self.spec = spec
        self.subdim = subdim
        self._build = build
        self._compiled = {}

    def compile(self, ver):
        if ver not in self._compiled:
            self._compiled[ver] = self._build(self.name, ver)
        return self._compiled[ver]


def _pg_reference(in0, in1, c0, c1, c2):
    Pp = in0.shape[0]
    S = in1.shape[-1] // 4
    x = in0.reshape(Pp, S, -1).astype(np.float32)
    bb = in1.reshape(Pp, S, 4).astype(np.float32)
    m = np.maximum(x[:, :, None, :], bb[:, :, :, None])
    acc = m.sum(axis=(1, 2, 3)) + c2 * bb.reshape(Pp, -1).sum(axis=1)
    return np.zeros((Pp, 1), np.float32), acc.reshape(Pp, 1)


def _register_pg_ops():
    names = ("RANK_PGMAX4", "RANK_PGMAX4C")
    if names[0] in _CACHE:
        return tuple(_CACHE[n] for n in names)
    if names[0] in dve_ops._SUB_OPCODE_FOR_NAME:
        for n in names:
            _CACHE[n] = next(o for o in dve_ops.OPS if o.name == n)
        return tuple(_CACHE[n] for n in names)
    meta = Spec(
        body=_spill_c3_to_src1(maxx(Src0, C0) + maxx(Src0, C1) + maxx(Src0, C3)),
        accum=_operator_add, accum_init=Zero, reference=_pg_reference)
    ops = []
    for name, carry in ((names[0], False), (names[1], True)):
        op = _HandOp(name, meta,
                     (lambda n, v, c=carry: _build_pg_uops(n, v, carry=c)),
                     subdim=True)
        row = 1 + len(dve_ops.OPS)
        assert row < 0x20
        dve_ops.OPS.append(op)
        dve_ops.CUSTOM_DVE_SPECS[op.name] = op.spec
        dve_ops._SUB_OPCODE_FOR_NAME[op.name] = row
        _CACHE[name] = op
        ops.append(op)
    return tuple(ops)


# --------------------------------------------------------------------------
# host-side layout prep


def _prepare(scores, labels):
    """Sort/compact all 1024 row-units. Returns dict of per-row arrays and
    the shared strata plan."""
    scores = np.ascontiguousarray(np.asarray(scores), dtype=np.float32)
    lab = np.ascontiguousarray(np.asarray(labels)).astype(np.float32)
    all_s = np.concatenate([scores, scores.T], axis=0)
    all_l = np.concatenate([lab, lab.T], axis=0)
    pos = all_l > 0.5
    rows = all_s.shape[0]

    npos = pos.sum(axis=1)
    nneg = all_s.shape[1] - npos
    wn = int(max(nneg.max(), npos.max()))

    a_desc = np.full((rows, wn), -LBIG, dtype=np.float16)
    b_list = []      # per row: fp16 pivots ascending (scalar K first removed)
    needs_list = []  # per row: needs of the DVE pivots (non-increasing)
    sc_needs = np.zeros((rows, K_SC), dtype=np.int64)
    bsc = np.full((rows, K_SC), LBIG, dtype=np.float32)

    for r in range(rows):
        # role A: stream = negatives+margin desc, pivots = positives asc
        avA = np.sort((all_s[r][~pos[r]] + MARGIN).astype(np.float16))
        bvA = np.sort(all_s[r][pos[r]].astype(np.float16))
        ndA = len(avA) - np.searchsorted(avA, bvA, side="right")
        # role B: stream = -positives desc, pivots = -(neg+margin) asc
        # (relu(a_j - b_i) == relu((-b_i) - (-a_j)))
        avB = np.sort((-all_s[r][pos[r]]).astype(np.float16))
        bvB = np.sort((-(all_s[r][~pos[r]] + MARGIN)).astype(np.float16))
        ndB = len(avB) - np.searchsorted(avB, bvB, side="right")
        sA = np.sort(ndA[ndA > 0])[::-1][K_SC:].sum()
        sB = np.sort(ndB[ndB > 0])[::-1][K_SC:].sum()
        av, bv, need = (avA, bvA, ndA) if sA <= sB else (avB, bvB, ndB)
        a_desc[r, :len(av)] = av[::-1]
        order = np.argsort(need, kind="stable")[::-1]  # deepest first
        bv, need = bv[order], need[order]
        k = min(K_SC, len(bv))
        bsc[r, :k] = -bv[:k].astype(np.float32)
        sc_needs[r, :k] = need[:k]
        bd, nd = bv[k:], need[k:]
        nz = nd > 0
        b_list.append(bd[nz])
        needs_list.append(nd[nz])

    # envelope m(t) = max over rows of #{DVE needs > t}
    t_arr = np.arange(wn + 1)
    m = np.zeros(wn + 1, dtype=np.int64)
    for nd in needs_list:
        if len(nd):
            cnt = (nd[:, None] > t_arr[None, :]).sum(0)
            np.maximum(m, cnt, out=m)

    # threshold DP -> strata [(S_groups, N_len)] in descending-N order
    g = np.ceil(m / 4.0).astype(np.int64)
    INF = float("inf")
    dp = np.full(wn + 1, INF)
    dp[wn] = 0.0
    parent = np.full(wn + 1, -1, dtype=np.int64)
    for t in range(wn - 1, -1, -1):
        best, bu = INF, -1
        for u in range(t + 1, wn + 1):
            if dp[u] == INF:
                continue
            ag = g[t] - g[u]
            c = dp[u] + ag * (u + 4) + (C_INSTR if ag > 0 else 0.0)
            if c < best:
                best, bu = c, u
        dp[t] = best
        parent[t] = bu
    strata = []  # descending N
    t = 0
    chain = []
    while t < wn and parent[t] != -1:
        u = parent[t]
        ag = int(g[t] - g[u])
        if ag:
            chain.append((ag, int(u)))
        t = u
    strata = chain[::-1]  # largest N first (covers deepest ranks)
    G = sum(s for s, _ in strata)

    # pack the DVE pivot stream rank-major (deepest first), pad with +LBIG
    b_rank = np.full((rows, 4 * G), LBIG, dtype=np.float16)
    for r in range(rows):
        bd = b_list[r]
        b_rank[r, :len(bd)] = bd

    # permute pivot columns into execution order (ascending N strata)
    offs_desc = np.cumsum([0] + [si for si, _ in strata])[:-1]
    exec_order = sorted(zip(strata, offs_desc), key=lambda z: z[0][1])
    perm = []
    exec_strata = []  # (S, N) ascending-N with contiguous exec layout
    for (si, ni), od in exec_order:
        perm.extend(range(4 * od, 4 * (od + si)))
        exec_strata.append((si, ni))
    b_dve = np.ascontiguousarray(b_rank[:, perm])

    # scalar column stream lengths (envelope over rows)
    sc_len = sc_needs.max(axis=0)  # [K_SC]

    cnt = (npos * nneg).astype(np.float64)
    valid = cnt > 0
    w = np.where(valid, 1.0 / np.maximum(cnt, 1.0), 0.0)
    aux = np.stack([w, valid.astype(np.float64)], axis=1).astype(np.float32)

    return dict(a=a_desc, b=b_dve, bsc=bsc, aux=aux, wn=wn, G=G,
                strata=tuple(exec_strata),
                sc_len=tuple(int(x) for x in sc_len))


# --------------------------------------------------------------------------
# device program


def _build_program(wn, G, strata, sc_len, carry=False, debug=False):
    """strata: exec-ordered (ascending N), contiguous column layout.

    Strata write their running accumulator to a reversed out AP so the final
    element of stratum idx lands at acct[:, K+idx]; no accum_out / reads /
    staircase needed.  Scalar accums land at acct[:, 0:K].  One reduce."""
    key = ("pg", wn, G, tuple(strata), tuple(sc_len), carry, debug)
    if key in _CACHE:
        return _CACHE[key]
    pg, pgc = _register_pg_ops()

    offs = []
    o = 0
    for si, ni in strata:
        offs.append(o)
        o += si
    exec_order = list(zip(strata, offs))
    n_str = len(strata)
    n_chunk = min(3, n_str)                     # strata covered by chunk 1
    n_first = exec_order[n_chunk - 1][0][1]     # a-prefix needed by them
    n_zero = exec_order[0][0][1]                # stratum-1 a-prefix
    first_S = sum(si for (si, _), _ in exec_order[:n_chunk])
    max_sn = max(si * ni for si, ni in strata)
    K = len(sc_len)
    AW = K + n_str + max_sn                     # acct width

    nc = bacc.Bacc("TRN2", target_bir_lowering=False, debug=False,
                   num_devices=N_CORES)
    a_in = nc.dram_tensor("a_blk", [P, wn], F16, kind="ExternalInput").ap()
    b_in = nc.dram_tensor("b_blk", [P, 4 * G], F16, kind="ExternalInput").ap()
    c_in = nc.dram_tensor("bsc_blk", [P, K], F32, kind="ExternalInput").ap()
    out = nc.dram_tensor("out", [P, 1], F32, kind="ExternalOutput").ap()

    a = nc.alloc_sbuf_tensor("a", [P, wn], F16).ap()
    b = nc.alloc_sbuf_tensor("b", [P, 4 * G], F16).ap()
    bsc = nc.alloc_sbuf_tensor("bsc", [P, K], F32).ap()
    acct = nc.alloc_sbuf_tensor("acct", [P, AW], F32).ap()
    trash_s = nc.alloc_sbuf_tensor("trash_s", [P, 1], F32).ap()
    tot = nc.alloc_sbuf_tensor("tot", [P, 1], F32).ap()

    with (
        nc.psum_tensor("trash_ps", [P, wn], F32) as trash_ps_h,
        nc.semaphore("s_a0") as s_a0,
        nc.semaphore("s_a1") as s_a1,
        nc.semaphore("s_a2") as s_a2,
        nc.semaphore("s_b1") as s_b1,
        nc.semaphore("s_b2") as s_b2,
        nc.semaphore("s_bsc") as s_bsc,
        nc.semaphore("s_act") as s_act,
        nc.semaphore("s_vv") as s_vv,
        nc.semaphore("s_v") as s_v,
        nc.Block() as block,
    ):
        @block.sync
        def _(sync):
            sync.dma_start(a[:, :n_zero], a_in[:, :n_zero]).then_inc(s_a0, 16)
            if n_first > n_zero:
                sync.dma_start(a[:, n_zero:n_first],
                               a_in[:, n_zero:n_first]).then_inc(s_a1, 16)
            else:
                sync.sem_inc(s_a1, 16)
            if wn > n_first:
                sync.dma_start(a[:, n_first:],
                               a_in[:, n_first:]).then_inc(s_a2, 16)
            else:
                sync.sem_inc(s_a2, 16)
            sync.wait_ge(s_v, 1)
            sync.dma_start(out[:], tot[:]).then_inc(s_v, 16)

        @block.gpsimd
        def _(gpsimd):
            if G > first_S:
                gpsimd.dma_start(b[:, 4 * first_S:],
                                 b_in[:, 4 * first_S:]).then_inc(s_b2, 16)
            else:
                gpsimd.sem_inc(s_b2, 16)

        @block.scalar
        def _(scalar):
            scalar.dma_start(b[:, :4 * first_S],
                             b_in[:, :4 * first_S]).then_inc(s_b1, 16)
            scalar.dma_start(bsc[:], c_in[:]).then_inc(s_bsc, 16)
            trash_ps = trash_ps_h.ap()
            # warm the Relu table while DMAs fly
            zc = nc.const_aps.scalar_like(0.0, trash_s[:, 0:1])
            scalar.activation(trash_s[:, 0:1], zc, AF.Relu, bias=0.0, scale=1.0)
            scalar.wait_ge(s_bsc, 16)
            scalar.wait_ge(s_a2, 16)
            for j in range(K):
                if j:
                    scalar.wait_ge(s_act, j)
                L = max(sc_len[j], 1)
                scalar.activation(trash_ps[:, :L], a[:, :L], AF.Relu,
                                  bias=bsc[:, j:j + 1], scale=1.0,
                                  accum_out=acct[:, j:j + 1]).then_inc(s_act, 1)

        @block.vector
        def _(vector):
            vector.wait_ge(s_a0, 16)
            vector.wait_ge(s_b1, 16)
            done_a1 = False
            done_a2 = False
            last = None
            for idx, ((S, N), off) in enumerate(exec_order):
                if idx == n_chunk:
                    vector.wait_ge(s_b2, 16)
                if not done_a1 and N > n_zero:
                    vector.wait_ge(s_a1, 16)
                    done_a1 = True
                if not done_a2 and N > n_first:
                    vector.wait_ge(s_a2, 16)
                    done_a2 = True
                in0 = a[:, :N].unsqueeze(1).broadcast_to((P, S, N))
                in1 = b[:, 4 * off:4 * (off + S)]
                hi = K + idx + S * N - 1
                last = vector._custom_dve(
                    pg, out=acct[:, hi:K + idx - 1:-1], in0=in0, in1=in1,
                    s0=0.0, s1=0.0, imm2=-float(N))
            last.then_inc(s_vv, 1)
            vector.wait_ge(s_vv, 1)
            vector.wait_ge(s_act, K)
            vector.reduce_sum(tot[:], acct[:, :K + n_str],
                              axis=mybir.AxisListType.X).then_inc(s_v, 1)

    nc.compile()
    _CACHE[key] = nc
    return nc


# --------------------------------------------------------------------------


def kernel(scores, labels):
    prep = _prepare(scores, labels)
    nc = _build_program(prep["wn"], prep["G"], tuple(prep["strata"]),
                        prep["sc_len"], carry=False)
    in_maps = [{"a_blk": prep["a"][P * k:P * (k + 1)],
                "b_blk": prep["b"][P * k:P * (k + 1)],
                "bsc_blk": prep["bsc"][P * k:P * (k + 1)]}
               for k in range(N_CORES)]
    res = run_bass_kernel_spmd(nc, in_maps, list(range(N_CORES)))
    tots = np.concatenate([res.results[k]["out"][:, 0]
                           for k in range(N_CORES)])
    w = prep["aux"][:, 0].astype(np.float64)
    valid = prep["aux"][:, 1].astype(np.float64)
    return np.float32((tots * w).sum() / valid.sum())
